# revision 26
# baseline (speedup 1.0000x reference)
"""Trainium2 Bass kernel for nn_Basic_Aggregator (gnn_message_passing).

Math: out[b, i, :] = sum_j node_j[b, j, :]  (sum over node axis, broadcast
back to every row).  edge_ij is unused by the computation.

Sharding: data-parallel over batch B=16 across 8 cores (2 batches/core).
Each core reads its [2, 20000, 64] slab, reduces each batch to a [64]
vector, broadcasts it back to [20000, 64] and writes it out.  No
cross-core communication.

Layout: 20000 rows = 125 partitions x 160 rows, so a whole batch is a
fully-contiguous [125, 10240] f32 slab (40960 B per partition).

DMA strategy (from trace analysis):
- A single dma_start is served by only 5 SDMA engines (descriptors
  split into 5 equal blocks over 5 consecutive engines); the SWDGE
  (gpsimd) path rotates the starting engine per instruction while the
  HWDGE queues pin to a fixed ~135 GB/s bundle.  All bulk transfers go
  through gpsimd as 8 uniform chunks per batch per direction (32 DMAs
  x 5 blocks = 10 blocks per engine) to load all 16 engines evenly.
- Measured per-NC caps here: reads ~224 GB/s, writes ~250 GB/s,
  reads+writes together ~356 GB/s.  The schedule aims for: b0 reads
  solo, then b1 reads duplexed with b0 stores, then b1 stores.
  Staggered gates (tiny Q7 copies, RAW on a later b0 chunk + WAW under
  each b1 load) keep b1's loads out of the rings until b0's reads are
  draining; otherwise the engines round-robin all queued loads and
  b0's sums (which gate the first stores) only complete near the end
  of ALL reads, serializing reads before writes.
Measured ~102-107 us/core vs 165 us for the HWDGE baseline;
20.48 MB/core of mandatory HBM traffic bounds this at ~75 us of
transfer plus ~20 us fixed prologue/drain overhead.
"""

import numpy as np

B, SIZE, D = 16, 20000, 64
N_CORES = 8
B_LOCAL = B // N_CORES  # 2
P = 125                 # partitions used; 125 * 160 = 20000 rows
NG = 160                # rows per partition
W = NG * D              # 10240 f32 per partition
CH = 8                  # uniform chunks per batch (loads and stores)
CW = W // CH            # 1280 f32 per partition per chunk (5120 B)

_STATE = {}

# Results of the most recent device run (for test harness introspection).
LAST_RESULT = None


def _patch_drain_split():
    """The walrus build in this container accepts at most one sync-wait
    command per instruction; Tile's kernel-tail drain collects one wait per
    dangling proc onto a single Drain.  Split it into a chain of
    single-wait drains on the same engine — identical semantics."""
    from concourse import tile
    import concourse.mybir as mybir
    from concourse.vector_clock import ScopedClock

    if getattr(tile.TileContext, "_ant_drain_split", False):
        return

    def _drain_and_barrier(self, tick_clock, wait_clock):
        drain_inst = self.nc.sync.drain()
        wait_clock.add_sem_waits(
            drain_inst.ins, ScopedClock({None: tick_clock.global_clock})
        )
        si = drain_inst.ins.sync_info
        if si is not None and si.on_wait and len(si.on_wait) > 1:
            waits = list(si.on_wait)
            upds = list(si.on_update or [])
            drain_inst.ins.sync_info = mybir.SyncInfo(
                on_wait=[waits[0]], on_update=[]
            )
            for i, w in enumerate(waits[1:]):
                extra = self.nc.sync.drain()
                extra.ins.sync_info = mybir.SyncInfo(
                    on_wait=[w],
                    on_update=upds if i == len(waits) - 2 else [],
                )

        self.nc.all_engine_barrier()
        assert self.sems is not None
        popped = self.nc._tile_sem_poison_stack.pop()
        assert popped is self._sem_poison
        self.nc.clear_and_free_semaphores(list(self.sems.allocated().values()))
        self.nc.all_engine_barrier()

    tile.TileContext._drain_and_barrier = _drain_and_barrier
    tile.TileContext._ant_drain_split = True

    # Same single-wait limitation, general case: any scheduled instruction
    # that picked up >1 sem waits (e.g. a DMA with both a cross-engine data
    # wait and a DMA-lane slot wait) is split — single-wait no-ops on the
    # same engine carry all but the last wait.  Identical semantics: the
    # sequencer blocks on each in order.
    orig_add = tile.TileContext._add_instruction

    def _add_instruction(self, inst):
        si = getattr(inst, "sync_info", None)
        if si is not None and si.on_wait and len(si.on_wait) > 1:
            waits = list(si.on_wait)
            for w in waits[:-1]:
                noop = mybir.InstNoOp(
                    name=self.nc.get_next_instruction_name(),
                    engine=inst.engine,
                    sync_info=mybir.SyncInfo(on_wait=[w], on_update=[]),
                    bass_nofuse=True,
                )
                orig_add(self, noop)
            inst.sync_info = mybir.SyncInfo(
                on_wait=[waits[-1]], on_update=list(si.on_update or [])
            )
        orig_add(self, inst)

    tile.TileContext._add_instruction = _add_instruction


def _build_nc():
    import concourse.bass as bass
    import concourse.mybir as mybir
    from concourse import tile

    _patch_drain_split()

    f32 = mybir.dt.float32
    nc = bass.Bass()
    x = nc.declare_dram_parameter("x", [B_LOCAL, SIZE, D], f32, isOutput=False)
    y = nc.declare_dram_parameter("y", [B_LOCAL, SIZE, D], f32, isOutput=True)

    with tile.TileContext(nc) as tc:
        with (
            tc.tile_pool(name="io", bufs=1) as io,
            tc.tile_pool(name="small", bufs=1) as small,
            tc.tile_pool(name="psum", bufs=2, space="PSUM") as psum,
        ):
            # all-ones [125,125]: one matmul both partition-reduces and
            # broadcasts: (ones.T @ part)[p, d] = sum_q part[q, d] for all p
            ones_sq = small.tile([P, P], f32, tag="ones_sq")
            nc.vector.memset(ones_sq[:], 1.0)

            # Loads: uniform chunks, mostly on the SWDGE (gpsimd) queue so
            # the per-instruction engine rotation tiles all 16 SDMA engines.
            # HBM reads alone cap at ~200 GB/s while reads+writes together
            # reach ~356 GB/s, so phase order matters: b0 loads get the read
            # bandwidth exclusively, then b1 loads run concurrently with b0
            # stores (duplex), then b1 stores drain.  Without a gate the
            # SDMA engines round-robin b0/b1 load descriptors and b0's last
            # chunk (which gates the first store) finishes near the end of
            # ALL loads, serializing reads before writes.
            chunks = {}

            def emit_load(b, c, gate_tile=None):
                t = io.tile([P, CW], f32, tag=f"in{b}_{c}")
                if gate_tile is not None:
                    # gate: RAW on the gate tile's producer, WAW under the
                    # load -> this load enters the DMA rings only after the
                    # producer finished (phase separation without starving
                    # the rings).
                    nc.gpsimd.tensor_copy(t[:, 0:1], gate_tile[:, 0:1])
                xb = x[b].rearrange("(p w) d -> p (w d)", p=P)
                nc.gpsimd.dma_start(out=t[:], in_=xb[:, c * CW:(c + 1) * CW])
                chunks[b, c] = t

            # Per-chunk reduce, PE accumulate+broadcast, widen, store in the
            # same uniform chunk geometry.
            def emit_compute_store(b):
                bc_psum = psum.tile([P, D], f32, tag="bc")
                for c in range(CH):
                    # contiguous in-place fold-adds (the strided
                    # reduce_sum view runs ~2x slower on DVE); the chunk
                    # tile is dead after this.
                    t = chunks[b, c][:]
                    w = CW
                    while w > 5 * D:
                        h = w // 2
                        nc.vector.tensor_add(t[:, 0:h], t[:, 0:h], t[:, h:w])
                        w = h
                    for k in range(1, w // D):
                        nc.vector.tensor_add(t[:, 0:D], t[:, 0:D],
                                             t[:, k * D:(k + 1) * D])
                    nc.tensor.matmul(bc_psum[:], ones_sq[:], t[:, 0:D],
                                     start=(c == 0), stop=(c == CH - 1))

                # widen bc_psum [125,64] to one chunk's width [125,1280]
                # (own pool: sharing the io ring adds a WAR wait on the load
                # DMAs to the first store, and this walrus build rejects >1
                # sync wait per instruction)
                wide = small.tile([P, CW], f32, tag=f"wide{b}")
                nc.vector.tensor_copy(wide[:, 0:D], bc_psum[:])
                w = D
                while w < CW:
                    cc = min(w, CW - w)
                    nc.vector.tensor_copy(wide[:, w:w + cc], wide[:, 0:cc])
                    w += cc

                # store: each chunk is a plain [125, 1280] copy of `wide`
                # (every output row within a batch is identical).  All
                # stores stay on SWDGE: routing some to the HWDGE rings was
                # tried and lost ~6us (the HWDGE bundle contends with SWDGE
                # for the same engines and its queue starts late).
                yb = y[b].rearrange("(p s w) d -> p s (w d)", p=P, s=CH)
                for c in range(CH):
                    nc.gpsimd.dma_start(out=yb[:, c], in_=wide[:])

            # SWDGE emission order decides both phase overlap and the
            # 8-lane round-robin sem assignment (one outstanding DMA per
            # lane).  Order [b0L x8][b1L 0-3][b0S x8][b1L 4-7][b1S x8]:
            # b1's stores inherit lane predecessors that are retired loads
            # or early b0 stores instead of stalling on the full b0 store
            # set at widen-b1 time.
            for c in range(CH):
                emit_load(0, c)
            for c in range(4):
                emit_load(1, c, chunks[0, min(c + 4, CH - 1)])
            emit_compute_store(0)
            for c in range(4, CH):
                emit_load(1, c, chunks[1, c - 4])
            emit_compute_store(1)

    return nc


def _get_nc():
    if "nc" not in _STATE:
        _STATE["nc"] = _build_nc()
    return _STATE["nc"]


def kernel(node_j, edge_ij=None):
    global LAST_RESULT
    from concourse.bass_utils import run_bass_kernel_spmd

    node_j = np.ascontiguousarray(np.asarray(node_j), dtype=np.float32)
    assert node_j.shape == (B, SIZE, D), node_j.shape

    nc = _get_nc()
    in_maps = [
        {"x": node_j[i * B_LOCAL:(i + 1) * B_LOCAL]} for i in range(N_CORES)
    ]
    res = run_bass_kernel_spmd(nc, in_maps, core_ids=list(range(N_CORES)))
    LAST_RESULT = res
    out = np.concatenate([r["y"] for r in res.results], axis=0)
    return out


# revision 27
# speedup vs baseline: 1.1370x; 1.1370x over previous
"""Trainium2 Bass kernel for nn_Basic_Aggregator (gnn_message_passing).

Math: out[b, i, :] = sum_j node_j[b, j, :]  (sum over node axis, broadcast
back to every row).  edge_ij is unused by the computation.

Sharding: data-parallel over batch B=16 across 8 cores (2 batches/core).
Each core reads its [2, 20000, 64] slab, reduces each batch to a [64]
vector, broadcasts it back to [20000, 64] and writes it out.  No
cross-core communication.

Layout: 20000 rows = 125 partitions x 160 rows, so a whole batch is a
fully-contiguous [125, 10240] f32 slab (40960 B per partition).

DMA strategy (from trace analysis):
- A single dma_start is served by only 5 SDMA engines (descriptors
  split into 5 equal blocks over 5 consecutive engines); the SWDGE
  (gpsimd) path rotates the starting engine per instruction while the
  HWDGE queues pin to a fixed ~135 GB/s bundle.  All bulk transfers go
  through gpsimd as 8 uniform chunks per batch per direction (32 DMAs
  x 5 blocks = 10 blocks per engine) to load all 16 engines evenly.
- Measured per-NC caps here: reads ~224 GB/s, writes ~250 GB/s,
  reads+writes together ~356 GB/s.  The schedule aims for: b0 reads
  solo, then b1 reads duplexed with b0 stores, then b1 stores.
  Staggered gates (tiny Q7 copies, RAW on a later b0 chunk + WAW under
  each b1 load) keep b1's loads out of the rings until b0's reads are
  draining; otherwise the engines round-robin all queued loads and
  b0's sums (which gate the first stores) only complete near the end
  of ALL reads, serializing reads before writes.
Measured ~102-107 us/core vs 165 us for the HWDGE baseline;
20.48 MB/core of mandatory HBM traffic bounds this at ~75 us of
transfer plus ~20 us fixed prologue/drain overhead.
"""

import numpy as np

B, SIZE, D = 16, 20000, 64
N_CORES = 8
B_LOCAL = B // N_CORES  # 2
P = 125                 # partitions used; 125 * 160 = 20000 rows
NG = 160                # rows per partition
W = NG * D              # 10240 f32 per partition
CH = 8                  # uniform chunks per batch (loads and stores)
CW = W // CH            # 1280 f32 per partition per chunk (5120 B)

_STATE = {}

# Results of the most recent device run (for test harness introspection).
LAST_RESULT = None


def _patch_drain_split():
    """The walrus build in this container accepts at most one sync-wait
    command per instruction; Tile's kernel-tail drain collects one wait per
    dangling proc onto a single Drain.  Split it into a chain of
    single-wait drains on the same engine — identical semantics."""
    from concourse import tile
    import concourse.mybir as mybir
    from concourse.vector_clock import ScopedClock

    if getattr(tile.TileContext, "_ant_drain_split", False):
        return

    def _drain_and_barrier(self, tick_clock, wait_clock):
        drain_inst = self.nc.sync.drain()
        wait_clock.add_sem_waits(
            drain_inst.ins, ScopedClock({None: tick_clock.global_clock})
        )
        si = drain_inst.ins.sync_info
        if si is not None and si.on_wait and len(si.on_wait) > 1:
            waits = list(si.on_wait)
            upds = list(si.on_update or [])
            drain_inst.ins.sync_info = mybir.SyncInfo(
                on_wait=[waits[0]], on_update=[]
            )
            for i, w in enumerate(waits[1:]):
                extra = self.nc.sync.drain()
                extra.ins.sync_info = mybir.SyncInfo(
                    on_wait=[w],
                    on_update=upds if i == len(waits) - 2 else [],
                )

        self.nc.all_engine_barrier()
        assert self.sems is not None
        popped = self.nc._tile_sem_poison_stack.pop()
        assert popped is self._sem_poison
        self.nc.clear_and_free_semaphores(list(self.sems.allocated().values()))
        self.nc.all_engine_barrier()

    tile.TileContext._drain_and_barrier = _drain_and_barrier
    tile.TileContext._ant_drain_split = True

    # Same single-wait limitation, general case: any scheduled instruction
    # that picked up >1 sem waits (e.g. a DMA with both a cross-engine data
    # wait and a DMA-lane slot wait) is split — single-wait no-ops on the
    # same engine carry all but the last wait.  Identical semantics: the
    # sequencer blocks on each in order.
    orig_add = tile.TileContext._add_instruction

    def _add_instruction(self, inst):
        si = getattr(inst, "sync_info", None)
        if si is not None and si.on_wait and len(si.on_wait) > 1:
            waits = list(si.on_wait)
            for w in waits[:-1]:
                noop = mybir.InstNoOp(
                    name=self.nc.get_next_instruction_name(),
                    engine=inst.engine,
                    sync_info=mybir.SyncInfo(on_wait=[w], on_update=[]),
                    bass_nofuse=True,
                )
                orig_add(self, noop)
            inst.sync_info = mybir.SyncInfo(
                on_wait=[waits[-1]], on_update=list(si.on_update or [])
            )
        orig_add(self, inst)

    tile.TileContext._add_instruction = _add_instruction


def _build_nc():
    import concourse.bass as bass
    import concourse.mybir as mybir
    from concourse import tile

    _patch_drain_split()

    f32 = mybir.dt.float32
    nc = bass.Bass()
    x = nc.declare_dram_parameter("x", [B_LOCAL, SIZE, D], f32, isOutput=False)
    y = nc.declare_dram_parameter("y", [B_LOCAL, SIZE, D], f32, isOutput=True)

    with tile.TileContext(nc) as tc:
        with (
            tc.tile_pool(name="io", bufs=1) as io,
            tc.tile_pool(name="small", bufs=1) as small,
            tc.tile_pool(name="psum", bufs=2, space="PSUM") as psum,
        ):
            # all-ones [125,125]: one matmul both partition-reduces and
            # broadcasts: (ones.T @ part)[p, d] = sum_q part[q, d] for all p
            ones_sq = small.tile([P, P], f32, tag="ones_sq")
            nc.vector.memset(ones_sq[:], 1.0)

            # Loads: uniform chunks, mostly on the SWDGE (gpsimd) queue so
            # the per-instruction engine rotation tiles all 16 SDMA engines.
            # HBM reads alone cap at ~200 GB/s while reads+writes together
            # reach ~356 GB/s, so phase order matters: b0 loads get the read
            # bandwidth exclusively, then b1 loads run concurrently with b0
            # stores (duplex), then b1 stores drain.  Without a gate the
            # SDMA engines round-robin b0/b1 load descriptors and b0's last
            # chunk (which gates the first store) finishes near the end of
            # ALL loads, serializing reads before writes.
            chunks = {}
            for b in range(B_LOCAL):
                xb = x[b].rearrange("(p w) d -> p (w d)", p=P)  # [125, 10240]
                for c in range(CH):
                    t = io.tile([P, CW], f32, tag=f"in{b}_{c}")
                    if b == 1:
                        # staggered gate: RAW on b0's chunk c+4 load, WAW
                        # under the b1 load -> b1 chunk c enters the DMA
                        # rings only as b0's reads are draining, keeping
                        # reads continuous without starving b0's tail.
                        g = chunks[0, min(c + 4, CH - 1)]
                        nc.gpsimd.tensor_copy(t[:, 0:1], g[:, 0:1])
                    nc.gpsimd.dma_start(out=t[:], in_=xb[:, c * CW:(c + 1) * CW])
                    chunks[b, c] = t

            # Per-chunk reduce, PE accumulate+broadcast, widen, store in the
            # same uniform chunk geometry.
            for b in range(B_LOCAL):
                bc_psum = psum.tile([P, D], f32, tag="bc")
                for c in range(CH):
                    # contiguous in-place fold-adds (the strided
                    # reduce_sum view runs ~2x slower on DVE); the chunk
                    # tile is dead after this.
                    t = chunks[b, c][:]
                    w = CW
                    while w > 5 * D:
                        h = w // 2
                        nc.vector.tensor_add(t[:, 0:h], t[:, 0:h], t[:, h:w])
                        w = h
                    for k in range(1, w // D):
                        nc.vector.tensor_add(t[:, 0:D], t[:, 0:D],
                                             t[:, k * D:(k + 1) * D])
                    nc.tensor.matmul(bc_psum[:], ones_sq[:], t[:, 0:D],
                                     start=(c == 0), stop=(c == CH - 1))

                # widen bc_psum [125,64] to one chunk's width [125,1280]
                # (own pool: sharing the io ring adds a WAR wait on the load
                # DMAs to the first store, and this walrus build rejects >1
                # sync wait per instruction)
                wide = small.tile([P, CW], f32, tag=f"wide{b}")
                nc.vector.tensor_copy(wide[:, 0:D], bc_psum[:])
                w = D
                while w < CW:
                    cc = min(w, CW - w)
                    nc.vector.tensor_copy(wide[:, w:w + cc], wide[:, 0:cc])
                    w += cc

                # store: each chunk is a plain [125, 1280] copy of `wide`
                # (every output row within a batch is identical).  All
                # stores stay on SWDGE: routing some to the HWDGE rings was
                # tried and lost ~6us (the HWDGE bundle contends with SWDGE
                # for the same engines and its queue starts late).
                yb = y[b].rearrange("(p s w) d -> p s (w d)", p=P, s=CH)
                for c in range(CH):
                    nc.gpsimd.dma_start(out=yb[:, c], in_=wide[:])

    return nc


def _get_nc():
    if "nc" not in _STATE:
        _STATE["nc"] = _build_nc()
    return _STATE["nc"]


def kernel(node_j, edge_ij=None):
    global LAST_RESULT
    from concourse.bass_utils import run_bass_kernel_spmd

    node_j = np.ascontiguousarray(np.asarray(node_j), dtype=np.float32)
    assert node_j.shape == (B, SIZE, D), node_j.shape

    nc = _get_nc()
    in_maps = [
        {"x": node_j[i * B_LOCAL:(i + 1) * B_LOCAL]} for i in range(N_CORES)
    ]
    res = run_bass_kernel_spmd(nc, in_maps, core_ids=list(range(N_CORES)))
    LAST_RESULT = res
    out = np.concatenate([r["y"] for r in res.results], axis=0)
    return out
